# revision 1
# baseline (speedup 1.0000x reference)
"""Multi-head attention (B=2, T=2048, C=1024, H=16, hd=64, RoPE, full mask)
on 8 TRN2 NeuronCores.

Sharding: tensor-parallel over (batch, head-group). Core c handles batch
c//4 and heads [4*(c%4) .. 4*(c%4)+3]. Each core computes the QKV
projection for its 4 heads, full attention over T=2048, and a partial
output projection y = o_heads @ w_proj[:, cols].T. The host sums the 4
partial y's per batch (the tensor-parallel unshard reduction).

On-chip layout is "transposed everything" so attention needs no on-chip
transposes of the big tensors:
  - qT/kT stored [head_dim, T] (projection computed as w @ x.T)
  - scores computed directly transposed: sT[tk, tq] = k[tk] . q[tq]
  - softmax denominator via an appended ones-column on V (M=66 matmul)
  - o.T scaled by 1/den via a small PE-transpose + batched reciprocal +
    0/1-matrix broadcast matmul
RoPE uses full-width elementwise ops after a host-side even/odd row
permutation of w_q/w_k plus one PE permutation matmul that swaps
adjacent 32-partition blocks.

Precision: projections (K=1024/256 contractions) run in float32r
(~1e-4); attention q/k/v/softmax run in bf16 with fp32 PSUM
accumulation. Heads are processed in pairs so each exp ACTIVATE covers
[128, 1024] (amortizes the ~200ns ACT op overhead).
"""

import ml_dtypes
import numpy as np

import concourse.bacc as bacc
import concourse.mybir as mybir
import concourse.tile as tile
from concourse.bass_utils import run_bass_kernel_spmd

# Problem constants (hardcoded per contract)
B, T, C = 2, 2048, 1024
N_HEAD = 16
HD = 64
N_CORES = 8
HPC = 4  # heads per core
GC = HPC * HD  # head channels per core = 256

P = 128
KC = C // P  # 8 contraction chunks for the projections
NQB = 4  # query blocks
TQ = T // NQB  # 512
NKB = T // P  # 16 key blocks
VW = HD + 2  # 66: v + ones col + pad col

F32 = mybir.dt.float32
F32R = mybir.dt.float32r
BF16 = mybir.dt.bfloat16
F16 = mybir.dt.float16

_PROGRAM = None


def _build_program():
    nc = bacc.Bacc(
        "TRN2", target_bir_lowering=False, debug=False, num_devices=N_CORES
    )

    xT_d = nc.dram_tensor("xT", [C, T], F16, kind="ExternalInput").ap()
    wqkT_d = nc.dram_tensor("wqkT", [C, 4 * P], F16, kind="ExternalInput").ap()
    wvT_d = nc.dram_tensor("wvT", [C, GC], F16, kind="ExternalInput").ap()
    wpT_d = nc.dram_tensor("wpT", [GC, C], F16, kind="ExternalInput").ap()
    cc_d = nc.dram_tensor("cc", [P, T], F16, kind="ExternalInput").ap()
    ss_d = nc.dram_tensor("ss", [P, T], F16, kind="ExternalInput").ap()
    psw_d = nc.dram_tensor("psw", [P, P], F16, kind="ExternalInput").ap()
    emat_d = nc.dram_tensor("emat", [HPC, 2 * P], F32R, kind="ExternalInput").ap()
    ident_d = nc.dram_tensor("ident", [P, P], F32, kind="ExternalInput").ap()
    esel_d = nc.dram_tensor("esel", [P, HPC], F32, kind="ExternalInput").ap()
    y_d = nc.dram_tensor("y", [T, C], F32, kind="ExternalOutput").ap()

    with tile.TileContext(nc) as tc:
        with (
            tc.tile_pool(name="consts", bufs=1) as consts,
            tc.tile_pool(name="bigs", bufs=1) as bigs,
            tc.tile_pool(name="tmps", bufs=2) as tmps,
            tc.tile_pool(name="expool", bufs=3) as expool,
            tc.tile_pool(name="psA", bufs=2, space="PSUM") as psA,
            tc.tile_pool(name="psB", bufs=2, space="PSUM") as psB,
            tc.tile_pool(name="psC", bufs=2, space="PSUM") as psC,
        ):
            # ---- resident loads ----
            x_k = []
            for kc in range(KC):
                t = bigs.tile([P, T], F16, tag=f"x{kc}", name=f"x{kc}")
                nc.sync.dma_start(out=t, in_=xT_d[kc * P : (kc + 1) * P, :])
                x_k.append(t)
            wqk_k = []
            for kc in range(KC):
                t = bigs.tile([P, 4 * P], F16, tag=f"wqk{kc}", name=f"wqk{kc}")
                nc.sync.dma_start(out=t, in_=wqkT_d[kc * P : (kc + 1) * P, :])
                wqk_k.append(t)
            wv_k = []
            for kc in range(KC):
                t = bigs.tile([P, GC], F16, tag=f"wv{kc}", name=f"wv{kc}")
                nc.sync.dma_start(out=t, in_=wvT_d[kc * P : (kc + 1) * P, :])
                wv_k.append(t)
            wp_k = []
            for kb in range(2):
                t = bigs.tile([P, C], F16, tag=f"wp{kb}", name=f"wp{kb}")
                nc.sync.dma_start(out=t, in_=wpT_d[kb * P : (kb + 1) * P, :])
                wp_k.append(t)
            cc_t = consts.tile([P, T], F16, tag="cc")
            nc.sync.dma_start(out=cc_t, in_=cc_d)
            ss_t = consts.tile([P, T], F16, tag="ss")
            nc.sync.dma_start(out=ss_t, in_=ss_d)
            psw_t = consts.tile([P, P], F16, tag="psw")
            nc.sync.dma_start(out=psw_t, in_=psw_d)
            emat_t = consts.tile([HPC, 2 * P], F32R, tag="emat")
            nc.sync.dma_start(out=emat_t, in_=emat_d)
            ident_t = consts.tile([P, P], F32, tag="ident")
            nc.sync.dma_start(out=ident_t, in_=ident_d)
            esel_t = consts.tile([P, HPC], F32, tag="esel")
            nc.sync.dma_start(out=esel_t, in_=esel_d)
            ones_f = consts.tile([P, 2 * HPC], F32, tag="ones_f")
            nc.vector.memset(ones_f, 1.0)

            # ---- phase 1a: q/k projection + RoPE (output bf16) ----
            # M-blocks: 0 -> heads 0,1 of q; 1 -> heads 2,3 of q;
            #           2 -> heads 0,1 of k; 3 -> heads 2,3 of k.
            # Within a block: [h_even rows(32); h_odd(32); h'_even; h'_odd]
            qk_sb = [
                bigs.tile([P, T], F16, tag=f"qk{mb}", name=f"qk{mb}")
                for mb in range(4)
            ]
            for n in range(NQB):
                ns = slice(n * TQ, (n + 1) * TQ)
                for mb in range(4):
                    ps = psA.tile([P, 2 * TQ], F32, tag="mmps", name="ps")
                    pss = ps[:, 0:TQ]
                    for kc in range(KC):
                        nc.tensor.matmul(
                            pss,
                            lhsT=wqk_k[kc][:, mb * P : (mb + 1) * P],
                            rhs=x_k[kc][:, ns],
                            start=(kc == 0),
                            stop=(kc == KC - 1),
                        )
                    sb = qk_sb[mb]
                    nc.vector.tensor_copy(sb[:, ns], pss)
                    sw = psB.tile([P, TQ], F32, tag="aux", name="sw")
                    nc.tensor.matmul(
                        sw, lhsT=psw_t, rhs=sb[:, ns], start=True, stop=True
                    )
                    nc.vector.tensor_mul(sb[:, ns], sb[:, ns], cc_t[:, ns])
                    tmp = tmps.tile([P, TQ], F16, tag="ropetmp")
                    nc.vector.tensor_mul(tmp, sw, ss_t[:, ns])
                    nc.vector.tensor_add(sb[:, ns], sb[:, ns], tmp)

            # ---- phase 1b: v projection into [T, 4*66] bf16, ones col ----
            va_list = []
            for tb in range(NKB):
                vp = psA.tile([P, 2 * TQ], F32, tag="mmps", name="vp")
                vps = vp[:, 0:GC]
                for kc in range(KC):
                    nc.tensor.matmul(
                        vps,
                        lhsT=x_k[kc][:, tb * P : (tb + 1) * P],
                        rhs=wv_k[kc],
                        start=(kc == 0),
                        stop=(kc == KC - 1),
                    )
                va = bigs.tile(
                    [P, HPC * VW], F16, tag=f"va{tb}", name=f"va{tb}"
                )
                va4 = va.rearrange("p (h c) -> p h c", c=VW)
                nc.vector.tensor_copy(
                    va4[:, :, HD : HD + 2],
                    ones_f.rearrange("p (h c) -> p h c", c=2),
                )
                nc.vector.tensor_copy(
                    va4[:, :, 0:HD], vps.rearrange("p (h c) -> p h c", c=HD)
                )
                va_list.append(va)

            # ---- phases 2+3, software-pipelined per query block:
            # attention(qb) runs, then finalize(qb-1) (reciprocal chain,
            # 1/den scaling, output projection) so the PE never idles
            # waiting on the normalization chain.
            pend = {}

            def attention(qb):
                qs = slice(qb * TQ, (qb + 1) * TQ)
                oevp = [
                    tmps.tile(
                        [P, TQ], F32, tag=f"oevp{p}", name=f"oevp{p}_{qb}",
                        bufs=2,
                    )
                    for p in range(2)
                ]
                den4 = tmps.tile(
                    [P, TQ], F32, tag="den4", name=f"den4_{qb}", bufs=2
                )
                nc.vector.memset(den4, 1.0)
                for p in range(2):
                    qt = qk_sb[p]
                    kt = qk_sb[2 + p]
                    oau = [
                        psC.tile([VW, TQ], F32, tag="oau", name=f"oau{i}")
                        for i in range(2)
                    ]
                    # software pipeline: AV lags QK/exp by one k-block so
                    # the PE never stalls waiting on the exp
                    exs = {}
                    for kb in range(NKB + 1):
                        if kb < NKB:
                            st2 = psA.tile(
                                [P, 2 * TQ], F32, tag="mmps", name="st2"
                            )
                            ks = slice(kb * P, (kb + 1) * P)
                            for i in range(2):
                                nc.tensor.matmul(
                                    st2[:, i * TQ : (i + 1) * TQ],
                                    lhsT=kt[i * HD : (i + 1) * HD, ks],
                                    rhs=qt[i * HD : (i + 1) * HD, qs],
                                    start=True,
                                    stop=True,
                                )
                            ex = expool.tile(
                                [P, 2 * TQ], F16, tag="ex", name="ex"
                            )
                            nc.scalar.activation(
                                out=ex,
                                in_=st2,
                                func=mybir.ActivationFunctionType.Exp,
                                scale=1.0 / np.sqrt(HD),
                            )
                            exs[kb] = ex
                        if kb >= 1:
                            pk = kb - 1
                            exp_prev = exs.pop(pk)
                            for i in range(2):
                                h = 2 * p + i
                                nc.tensor.matmul(
                                    oau[i],
                                    lhsT=va_list[pk][:, h * VW : h * VW + VW],
                                    rhs=exp_prev[:, i * TQ : (i + 1) * TQ],
                                    start=(pk == 0),
                                    stop=(pk == NKB - 1),
                                )
                    # stage o (unnormalized) and the denominators
                    for i in range(2):
                        nc.vector.tensor_copy(
                            oevp[p][i * HD : (i + 1) * HD, :], oau[i][0:HD, :]
                        )
                        r = 32 * (2 * p + i)
                        nc.vector.tensor_copy(
                            den4[r : r + 1, :], oau[i][HD : HD + 1, :]
                        )
                pend[qb] = (oevp, den4)

            def finalize(qb):
                oevp, den4 = pend.pop(qb)
                o_sb = [
                    tmps.tile(
                        [P, TQ], F16, tag=f"osb{p}", name=f"osb{p}_{qb}",
                        bufs=2,
                    )
                    for p in range(2)
                ]
                # batched reciprocal: gather dens to [128, 16] via a
                # selection matmul, one reciprocal, transpose back,
                # broadcast via 0/1 matmul
                denT = psB.tile([P, 4 * HPC], F32, tag="aux", name="denT")
                for c in range(4):
                    nc.tensor.matmul(
                        denT[:, c * HPC : (c + 1) * HPC],
                        lhsT=den4[:, c * P : (c + 1) * P],
                        rhs=esel_t,
                        start=True,
                        stop=True,
                    )
                rdenT = tmps.tile([P, 4 * HPC], F32, tag="rdenT")
                nc.vector.reciprocal(rdenT, denT)
                rden_ps = psB.tile([HPC, TQ], F32, tag="aux", name="rden_ps")
                for c in range(4):
                    nc.tensor.transpose(
                        rden_ps[:, c * P : (c + 1) * P],
                        rdenT[:, c * HPC : (c + 1) * HPC],
                        ident_t,
                    )
                rden4 = tmps.tile([HPC, TQ], F32R, tag="rden4")
                with nc.allow_low_precision(reason="f32r round of 1/den"):
                    nc.vector.tensor_copy(rden4, rden_ps)
                for p in range(2):
                    bc = psB.tile([P, TQ], F32, tag="aux", name="bc")
                    nc.tensor.matmul(
                        bc,
                        lhsT=emat_t[:, p * P : (p + 1) * P],
                        rhs=rden4,
                        start=True,
                        stop=True,
                    )
                    nc.vector.tensor_mul(o_sb[p], oevp[p], bc)

                # output projection for this query block: y rows qb*512..
                for tch in range(TQ // P):
                    for cch in range(C // TQ):
                        yp = psB.tile([P, TQ], F32, tag="aux", name="yp")
                        for kb in range(2):
                            nc.tensor.matmul(
                                yp,
                                lhsT=o_sb[kb][:, tch * P : (tch + 1) * P],
                                rhs=wp_k[kb][:, cch * TQ : (cch + 1) * TQ],
                                start=(kb == 0),
                                stop=(kb == 1),
                            )
                        ysb = tmps.tile([P, TQ], F32, tag="ysb")
                        nc.vector.tensor_copy(ysb, yp)
                        r0 = qb * TQ + tch * P
                        nc.sync.dma_start(
                            out=y_d[r0 : r0 + P, cch * TQ : (cch + 1) * TQ],
                            in_=ysb,
                        )

            for qb in range(NQB):
                attention(qb)
                if qb >= 1:
                    finalize(qb - 1)
            finalize(NQB - 1)

    nc.compile()
    return nc


def _get_program():
    global _PROGRAM
    if _PROGRAM is None:
        _PROGRAM = _build_program()
    return _PROGRAM


def _eo(w):
    """[64, C] head rows -> [even(32); odd(32)]"""
    return np.concatenate([w[0::2], w[1::2]], axis=0)


def _host_prep(x, cos, sin, w_qkv, w_proj):
    """Build the 8 per-core input maps."""
    f16 = np.float16
    xT = [np.ascontiguousarray(x[b].T).astype(f16) for b in range(B)]  # [C, T]

    cosT = np.ascontiguousarray(cos.T)  # [32, T]
    sinT = np.ascontiguousarray(sin.T)
    cc = np.tile(cosT, (4, 1)).astype(f16)  # [128, T]
    ss = np.tile(np.concatenate([-sinT, sinT], axis=0), (2, 1)).astype(f16)
    psw = np.zeros((P, P), dtype=np.float32)
    idx = np.arange(P)
    psw[idx, idx ^ 32] = 1.0
    psw = psw.astype(f16)
    emat = np.zeros((HPC, 2 * P), dtype=np.float32)
    for p in range(2):
        for i in range(2):
            emat[2 * p + i, p * P + i * HD : p * P + (i + 1) * HD] = 1.0
    ident = np.eye(P, dtype=np.float32)
    esel = np.zeros((P, HPC), dtype=np.float32)
    for j in range(HPC):
        esel[32 * j, j] = 1.0

    wq = w_qkv[0:C]
    wk = w_qkv[C : 2 * C]
    wv = w_qkv[2 * C : 3 * C]

    in_maps = []
    for core in range(N_CORES):
        b = core // 4
        h0 = 4 * (core % 4)
        heads = [h0, h0 + 1, h0 + 2, h0 + 3]
        blocks = []
        for pair in range(2):
            ha, hb = heads[2 * pair], heads[2 * pair + 1]
            blocks.append(
                np.concatenate(
                    [_eo(wq[ha * HD : ha * HD + HD]),
                     _eo(wq[hb * HD : hb * HD + HD])],
                    axis=0,
                )
            )
        for pair in range(2):
            ha, hb = heads[2 * pair], heads[2 * pair + 1]
            blocks.append(
                np.concatenate(
                    [_eo(wk[ha * HD : ha * HD + HD]),
                     _eo(wk[hb * HD : hb * HD + HD])],
                    axis=0,
                )
            )
        wqkT = np.ascontiguousarray(
            np.concatenate(blocks, axis=0).T
        ).astype(f16)  # [C, 512]
        wvT = np.ascontiguousarray(
            wv[h0 * HD : h0 * HD + GC].T
        ).astype(f16)  # [C, 256]
        wpT = np.ascontiguousarray(
            w_proj[:, h0 * HD : h0 * HD + GC].T
        ).astype(f16)  # [256, C]
        in_maps.append(
            {
                "xT": xT[b],
                "wqkT": wqkT,
                "wvT": wvT,
                "wpT": wpT,
                "cc": cc,
                "ss": ss,
                "psw": psw,
                "emat": emat,
                "ident": ident,
                "esel": esel,
            }
        )
    return in_maps


def kernel(x, cos, sin, mask, w_qkv, w_proj, _trace=False, _tmpdir=None):
    x = np.asarray(x, dtype=np.float32)
    cos = np.asarray(cos, dtype=np.float32)
    sin = np.asarray(sin, dtype=np.float32)
    w_qkv = np.asarray(w_qkv, dtype=np.float32)
    w_proj = np.asarray(w_proj, dtype=np.float32)
    # mask is all-ones in this problem spec: no-op in the math.

    nc = _get_program()
    in_maps = _host_prep(x, cos, sin, w_qkv, w_proj)
    res = run_bass_kernel_spmd(
        nc, in_maps, list(range(N_CORES)), trace=_trace, tmpdir=_tmpdir
    )
    out = np.empty((B, T, C), dtype=np.float32)
    for b in range(B):
        acc = res.results[4 * b]["y"].astype(np.float32).copy()
        for g in range(1, 4):
            acc += res.results[4 * b + g]["y"]
        out[b] = acc
    kernel._last_exec_time_ns = res.exec_time_ns
    return out



# revision 2
# speedup vs baseline: 1.0227x; 1.0227x over previous
"""Multi-head attention (B=2, T=2048, C=1024, H=16, hd=64, RoPE, full mask)
on 8 TRN2 NeuronCores.

Sharding: tensor-parallel over (batch, head-group). Core c handles batch
c//4 and heads [4*(c%4) .. 4*(c%4)+3]. Each core computes the QKV
projection for its 4 heads, full attention over T=2048, and a partial
output projection y = o_heads @ w_proj[:, cols].T. The host sums the 4
partial y's per batch (the tensor-parallel unshard reduction).

On-chip layout is "transposed everything" so attention needs no on-chip
transposes of the big tensors:
  - qT/kT stored [head_dim, T] (projection computed as w @ x.T)
  - scores computed directly transposed: sT[tk, tq] = k[tk] . q[tq]
  - softmax denominator via an appended ones-column on V (M=66 matmul)
  - o.T scaled by 1/den via a small PE-transpose + batched reciprocal +
    0/1-matrix broadcast matmul
RoPE uses full-width elementwise ops after a host-side even/odd row
permutation of w_q/w_k plus one PE permutation matmul that swaps
adjacent 32-partition blocks.

Schedule: the scalar-engine exp over the T^2 scores (~142us/core) is the
binding resource, so the program is one continuous exp-paced pipeline:
  - priority-ordered slab DMAs (k-weights, x[n0], rope tables, q-weights,
    v-weights, x[n1:]) so the first projection starts ~3.5us in;
  - warmup matmuls ramp the PE p-state during the DMA wait and a scale=0
    exp preloads the ACT table (its output doubles as the ones constant);
  - prologue projects only k(n0), q(qb0), v(tb0); every remaining
    projection, RoPE, finalize and output-projection chunk is a "filler"
    unit pumped between attention ticks with per-unit deadlines, keeping
    the PE busy inside the exp-bound window instead of serializing
    before/after it.

Precision: f16 operands with fp32 PSUM accumulation everywhere.
"""

import heapq

import ml_dtypes  # noqa: F401
import numpy as np

import concourse.bacc as bacc
import concourse.mybir as mybir
import concourse.tile as tile
from concourse.bass_utils import run_bass_kernel_spmd

# Problem constants (hardcoded per contract)
B, T, C = 2, 2048, 1024
N_HEAD = 16
HD = 64
N_CORES = 8
HPC = 4  # heads per core
GC = HPC * HD  # head channels per core = 256

P = 128
KC = C // P  # 8 contraction chunks for the projections
NQB = 4  # query blocks
TQ = T // NQB  # 512
NKB = T // P  # 16 key blocks
VW = HD + 2  # 66: v + ones col + pad col

F32 = mybir.dt.float32
F32R = mybir.dt.float32r
F16 = mybir.dt.float16

_PROGRAM = None


def _build_program():
    nc = bacc.Bacc(
        "TRN2", target_bir_lowering=False, debug=False, num_devices=N_CORES
    )

    xT_d = nc.dram_tensor("xT", [C, T], F16, kind="ExternalInput").ap()
    wqkT_d = nc.dram_tensor("wqkT", [C, 4 * P], F16, kind="ExternalInput").ap()
    wvT_d = nc.dram_tensor("wvT", [C, GC], F16, kind="ExternalInput").ap()
    wpT_d = nc.dram_tensor("wpT", [GC, C], F16, kind="ExternalInput").ap()
    cc_d = nc.dram_tensor("cc", [P, T], F16, kind="ExternalInput").ap()
    ss_d = nc.dram_tensor("ss", [P, T], F16, kind="ExternalInput").ap()
    psw_d = nc.dram_tensor("psw", [P, P], F16, kind="ExternalInput").ap()
    emat_d = nc.dram_tensor("emat", [HPC, 2 * P], F32R, kind="ExternalInput").ap()
    ident_d = nc.dram_tensor("ident", [P, P], F32, kind="ExternalInput").ap()
    esel_d = nc.dram_tensor("esel", [P, HPC], F32, kind="ExternalInput").ap()
    y_d = nc.dram_tensor("y", [T, C], F32, kind="ExternalOutput").ap()

    with tile.TileContext(nc) as tc:
        with (
            tc.tile_pool(name="consts", bufs=1) as consts,
            tc.tile_pool(name="bigs", bufs=1) as bigs,
            tc.tile_pool(name="tmps", bufs=2) as tmps,
            tc.tile_pool(name="expool", bufs=3) as expool,
            tc.tile_pool(name="psA", bufs=2, space="PSUM") as psA,
            tc.tile_pool(name="psB", bufs=2, space="PSUM") as psB,
            tc.tile_pool(name="psC", bufs=2, space="PSUM") as psC,
        ):
            # ---- resident tiles ----
            x_big = bigs.tile([P, KC * T], F16, tag="xbig", name="xbig")
            x3 = x_big.rearrange("p (kc t) -> p kc t", t=T)
            wqk_big = bigs.tile([P, KC * 4 * P], F16, tag="wqkbig", name="wqkbig")
            wqk3 = wqk_big.rearrange("p (kc m) -> p kc m", m=4 * P)
            wv_big = bigs.tile([P, KC * GC], F16, tag="wvbig", name="wvbig")
            wv3 = wv_big.rearrange("p (kc m) -> p kc m", m=GC)
            wp_big = bigs.tile([P, 2 * C], F16, tag="wpbig", name="wpbig")
            wp3 = wp_big.rearrange("p (kb m) -> p kb m", m=C)
            cc_t = consts.tile([P, T], F16, tag="cc")
            ss_t = consts.tile([P, T], F16, tag="ss")
            psw_t = consts.tile([P, P], F16, tag="psw")
            emat_t = consts.tile([HPC, 2 * P], F32R, tag="emat")
            ident_t = consts.tile([P, P], F32, tag="ident")
            esel_t = consts.tile([P, HPC], F32, tag="esel")

            xsrc = xT_d.rearrange("(kc p) t -> p kc t", p=P)
            wqksrc = wqkT_d.rearrange("(kc p) m -> p kc m", p=P)
            wvsrc = wvT_d.rearrange("(kc p) m -> p kc m", p=P)
            wpsrc = wpT_d.rearrange("(kb p) m -> p kb m", p=P)

            # ---- warmup: ramp the PE p-state during the DMA wait and
            # preload the ACT exp table. exp(0*x)=1 makes the ones tile.
            warm = consts.tile([P, TQ], F16, tag="warm")
            nc.vector.memset(warm, 0.0)
            wps = psA.tile([P, 2 * TQ], F32, tag="mmps", name="warmps")
            for i in range(10):
                nc.tensor.matmul(
                    wps[:, 0:TQ],
                    lhsT=warm[:, 0:P],
                    rhs=warm,
                    start=(i == 0),
                    stop=(i == 9),
                )
            ones_f = consts.tile([P, TQ], F32, tag="ones_f")
            nc.scalar.activation(
                out=ones_f,
                in_=wps[:, 0:TQ],
                func=mybir.ActivationFunctionType.Exp,
                scale=0.0,
            )
            ones4 = ones_f[:, 0 : 2 * HPC].rearrange("p (h c) -> p h c", c=2)

            # ---- DMAs in priority order (deps of early compute first) ----
            nc.sync.dma_start(
                out=wqk3[:, :, 2 * P : 4 * P], in_=wqksrc[:, :, 2 * P : 4 * P]
            )  # k weights
            nc.sync.dma_start(out=x3[:, :, 0:TQ], in_=xsrc[:, :, 0:TQ])  # x n0
            nc.sync.dma_start(out=cc_t[:, 0:TQ], in_=cc_d[:, 0:TQ])
            nc.sync.dma_start(out=ss_t[:, 0:TQ], in_=ss_d[:, 0:TQ])
            nc.sync.dma_start(out=psw_t, in_=psw_d)
            nc.sync.dma_start(
                out=wqk3[:, :, 0 : 2 * P], in_=wqksrc[:, :, 0 : 2 * P]
            )  # q weights
            nc.sync.dma_start(out=wv3, in_=wvsrc)
            nc.sync.dma_start(out=x3[:, :, TQ:T], in_=xsrc[:, :, TQ:T])  # x n1-3
            nc.sync.dma_start(out=cc_t[:, TQ:T], in_=cc_d[:, TQ:T])
            nc.sync.dma_start(out=ss_t[:, TQ:T], in_=ss_d[:, TQ:T])
            nc.sync.dma_start(out=wp3, in_=wpsrc)
            nc.sync.dma_start(out=emat_t, in_=emat_d)
            nc.sync.dma_start(out=ident_t, in_=ident_d)
            nc.sync.dma_start(out=esel_t, in_=esel_d)

            qk_sb = [
                bigs.tile([P, T], F16, tag=f"qk{mb}", name=f"qk{mb}")
                for mb in range(4)
            ]
            va_list = [
                bigs.tile([P, HPC * VW], F16, tag=f"va{tb}", name=f"va{tb}")
                for tb in range(NKB)
            ]

            # ---- work-unit emitters ----
            def proj_qk_tile(mb, n):
                """qk projection tile (mb, n) + RoPE. ~2.1us of PE."""
                ns = slice(n * TQ, (n + 1) * TQ)
                ps = psA.tile([P, 2 * TQ], F32, tag="mmps", name=f"ps{mb}_{n}")
                pss = ps[:, 0:TQ]
                for kc in range(KC):
                    nc.tensor.matmul(
                        pss,
                        lhsT=wqk_big[:, kc * 4 * P + mb * P : kc * 4 * P + (mb + 1) * P],
                        rhs=x_big[:, kc * T + n * TQ : kc * T + (n + 1) * TQ],
                        start=(kc == 0),
                        stop=(kc == KC - 1),
                    )
                sb = qk_sb[mb]
                nc.vector.tensor_copy(sb[:, ns], pss)
                sw = psB.tile([P, TQ], F32, tag="aux", name=f"sw{mb}_{n}")
                nc.tensor.matmul(sw, lhsT=psw_t, rhs=sb[:, ns], start=True, stop=True)
                nc.vector.tensor_mul(sb[:, ns], sb[:, ns], cc_t[:, ns])
                tmp = tmps.tile([P, TQ], F16, tag="ropetmp")
                nc.vector.tensor_mul(tmp, sw, ss_t[:, ns])
                nc.vector.tensor_add(sb[:, ns], sb[:, ns], tmp)

            def proj_v_tb(tb):
                """v projection for one 128-token block. ~1.2us of PE."""
                vp = psA.tile([P, 2 * TQ], F32, tag="mmps", name=f"vp{tb}")
                vps = vp[:, 0:GC]
                for kc in range(KC):
                    nc.tensor.matmul(
                        vps,
                        lhsT=x_big[:, kc * T + tb * P : kc * T + (tb + 1) * P],
                        rhs=wv_big[:, kc * GC : (kc + 1) * GC],
                        start=(kc == 0),
                        stop=(kc == KC - 1),
                    )
                va = va_list[tb]
                va4 = va.rearrange("p (h c) -> p h c", c=VW)
                nc.vector.tensor_copy(va4[:, :, HD : HD + 2], ones4)
                nc.vector.tensor_copy(
                    va4[:, :, 0:HD], vps.rearrange("p (h c) -> p h c", c=HD)
                )

            # ---- deadline-scheduled filler pump ----
            # Ticks g = qb*32 + pair*16 + kb pace the exp pipeline; filler
            # units (cost_ns, fn) are force-pumped at their deadline tick
            # and opportunistically earlier when the PE has spare credit.
            fillers = []  # heap of (deadline, seq, cost, fn)
            fseq = [0]
            credit = [0.0]
            ACT_NS = 1113.0
            BASE_NS = 680.0

            def add_filler(deadline, cost, fn):
                heapq.heappush(fillers, (deadline, fseq[0], cost, fn))
                fseq[0] += 1

            def tick(g):
                credit[0] = min(credit[0] + (ACT_NS - BASE_NS), 2600.0)
                while fillers and fillers[0][0] <= g:
                    _, _, c, fn = heapq.heappop(fillers)
                    fn()
                    credit[0] -= c
                credit[0] = max(credit[0], -1800.0)
                while fillers and credit[0] >= fillers[0][2]:
                    _, _, c, fn = heapq.heappop(fillers)
                    fn()
                    credit[0] -= c

            def drain_fillers():
                while fillers:
                    _, _, _, fn = heapq.heappop(fillers)
                    fn()

            # ---- attention sweep for one query block ----
            pend = {}

            def attention(qb):
                qs = slice(qb * TQ, (qb + 1) * TQ)
                oevp = [
                    tmps.tile(
                        [P, TQ], F32, tag=f"oevp{p}", name=f"oevp{p}_{qb}", bufs=2
                    )
                    for p in range(2)
                ]
                den4 = tmps.tile([P, TQ], F32, tag="den4", name=f"den4_{qb}", bufs=2)
                nc.vector.memset(den4, 1.0)
                for p in range(2):
                    qt = qk_sb[p]
                    kt = qk_sb[2 + p]
                    oau = [
                        psC.tile([VW, TQ], F32, tag="oau", name=f"oau{i}")
                        for i in range(2)
                    ]
                    # software pipeline: AV lags QK/exp by one k-block
                    exs = {}
                    for kb in range(NKB + 1):
                        tick(qb * 32 + p * 16 + min(kb, NKB - 1))
                        if kb < NKB:
                            st2 = psA.tile([P, 2 * TQ], F32, tag="mmps", name="st2")
                            ks = slice(kb * P, (kb + 1) * P)
                            for i in range(2):
                                nc.tensor.matmul(
                                    st2[:, i * TQ : (i + 1) * TQ],
                                    lhsT=kt[i * HD : (i + 1) * HD, ks],
                                    rhs=qt[i * HD : (i + 1) * HD, qs],
                                    start=True,
                                    stop=True,
                                )
                            ex = expool.tile([P, 2 * TQ], F16, tag="ex", name="ex")
                            nc.scalar.activation(
                                out=ex,
                                in_=st2,
                                func=mybir.ActivationFunctionType.Exp,
                                scale=1.0 / np.sqrt(HD),
                            )
                            exs[kb] = ex
                        if kb >= 1:
                            pk = kb - 1
                            exp_prev = exs.pop(pk)
                            for i in range(2):
                                h = 2 * p + i
                                nc.tensor.matmul(
                                    oau[i],
                                    lhsT=va_list[pk][:, h * VW : h * VW + VW],
                                    rhs=exp_prev[:, i * TQ : (i + 1) * TQ],
                                    start=(pk == 0),
                                    stop=(pk == NKB - 1),
                                )
                    # stage o (unnormalized) and the denominators
                    for i in range(2):
                        nc.vector.tensor_copy(
                            oevp[p][i * HD : (i + 1) * HD, :], oau[i][0:HD, :]
                        )
                        r = 32 * (2 * p + i)
                        nc.vector.tensor_copy(
                            den4[r : r + 1, :], oau[i][HD : HD + 1, :]
                        )
                pend[qb] = (oevp, den4)

            # ---- finalize units: reciprocal chain then y projection ----
            def fin_chain(qb):
                oevp, den4 = pend.pop(qb)
                o_sb = [
                    tmps.tile(
                        [P, TQ], F16, tag=f"osb{p}", name=f"osb{p}_{qb}", bufs=2
                    )
                    for p in range(2)
                ]
                denT = psB.tile([P, 4 * HPC], F32, tag="aux", name="denT")
                for c in range(4):
                    nc.tensor.matmul(
                        denT[:, c * HPC : (c + 1) * HPC],
                        lhsT=den4[:, c * P : (c + 1) * P],
                        rhs=esel_t,
                        start=True,
                        stop=True,
                    )
                rdenT = tmps.tile([P, 4 * HPC], F32, tag="rdenT")
                nc.vector.reciprocal(rdenT, denT)
                rden_ps = psB.tile([HPC, TQ], F32, tag="aux", name="rden_ps")
                for c in range(4):
                    nc.tensor.transpose(
                        rden_ps[:, c * P : (c + 1) * P],
                        rdenT[:, c * HPC : (c + 1) * HPC],
                        ident_t,
                    )
                rden4 = tmps.tile([HPC, TQ], F32R, tag="rden4")
                with nc.allow_low_precision(reason="f32r round of 1/den"):
                    nc.vector.tensor_copy(rden4, rden_ps)
                for p in range(2):
                    bc = psB.tile([P, TQ], F32, tag="aux", name="bc")
                    nc.tensor.matmul(
                        bc,
                        lhsT=emat_t[:, p * P : (p + 1) * P],
                        rhs=rden4,
                        start=True,
                        stop=True,
                    )
                    nc.vector.tensor_mul(o_sb[p], oevp[p], bc)
                return o_sb

            def yproj_tch(qb, o_sb, tch):
                for cch in range(C // TQ):
                    yp = psB.tile([P, TQ], F32, tag="aux", name="yp")
                    for kb in range(2):
                        nc.tensor.matmul(
                            yp,
                            lhsT=o_sb[kb][:, tch * P : (tch + 1) * P],
                            rhs=wp_big[:, kb * C + cch * TQ : kb * C + (cch + 1) * TQ],
                            start=(kb == 0),
                            stop=(kb == 1),
                        )
                    ysb = tmps.tile([P, TQ], F32, tag="ysb")
                    nc.vector.tensor_copy(ysb, yp)
                    r0 = qb * TQ + tch * P
                    nc.sync.dma_start(
                        out=y_d[r0 : r0 + P, cch * TQ : (cch + 1) * TQ],
                        in_=ysb,
                    )

            def add_finalize_fillers(qb):
                base = qb * 32 + 32
                o_box = {}

                def f1():
                    o_box["o"] = fin_chain(qb)

                add_filler(base + 2, 1800.0, f1)
                for tch in range(TQ // P):
                    add_filler(
                        base + 6 + 5 * tch,
                        1300.0,
                        lambda t=tch: yproj_tch(qb, o_box["o"], t),
                    )

            # ---- prologue: k(n0), q(qb0), v(tb0) ----
            proj_qk_tile(2, 0)
            proj_qk_tile(0, 0)
            proj_v_tb(0)
            proj_qk_tile(3, 0)
            proj_qk_tile(1, 0)

            # ---- seed filler units with deadlines ----
            # v(tb): needed by AV(qb0, p0, kb=tb) emitted at tick tb+1.
            for tb in range(1, NKB):
                add_filler(tb, 1250.0, lambda t=tb: proj_v_tb(t))
            # k tiles: pair0 needs (mb2, nb) at tick 4nb; pair1 (mb3, nb)
            # at tick 16+4nb.
            for nb in range(1, NQB):
                add_filler(4 * nb - 1, 2100.0, lambda n=nb: proj_qk_tile(2, n))
                add_filler(16 + 4 * nb - 2, 2100.0, lambda n=nb: proj_qk_tile(3, n))
            # q tiles: qb=nb pair0 needs (mb0, nb) at tick 32nb; pair1
            # (mb1, nb) at 32nb+16.
            for nb in range(1, NQB):
                add_filler(32 * nb - 6, 2100.0, lambda n=nb: proj_qk_tile(0, n))
                add_filler(32 * nb + 10, 2100.0, lambda n=nb: proj_qk_tile(1, n))

            # ---- main pipeline ----
            for qb in range(NQB):
                attention(qb)
                add_finalize_fillers(qb)
            drain_fillers()

    nc.compile()
    return nc


def _get_program():
    global _PROGRAM
    if _PROGRAM is None:
        _PROGRAM = _build_program()
    return _PROGRAM


def _eo(w):
    """[64, C] head rows -> [even(32); odd(32)]"""
    return np.concatenate([w[0::2], w[1::2]], axis=0)


def _host_prep(x, cos, sin, w_qkv, w_proj):
    """Build the 8 per-core input maps."""
    f16 = np.float16
    xT = [np.ascontiguousarray(x[b].T).astype(f16) for b in range(B)]  # [C, T]

    cosT = np.ascontiguousarray(cos.T)  # [32, T]
    sinT = np.ascontiguousarray(sin.T)
    cc = np.tile(cosT, (4, 1)).astype(f16)  # [128, T]
    ss = np.tile(np.concatenate([-sinT, sinT], axis=0), (2, 1)).astype(f16)
    psw = np.zeros((P, P), dtype=np.float32)
    idx = np.arange(P)
    psw[idx, idx ^ 32] = 1.0
    psw = psw.astype(f16)
    emat = np.zeros((HPC, 2 * P), dtype=np.float32)
    for p in range(2):
        for i in range(2):
            emat[2 * p + i, p * P + i * HD : p * P + (i + 1) * HD] = 1.0
    ident = np.eye(P, dtype=np.float32)
    esel = np.zeros((P, HPC), dtype=np.float32)
    for j in range(HPC):
        esel[32 * j, j] = 1.0

    wq = w_qkv[0:C]
    wk = w_qkv[C : 2 * C]
    wv = w_qkv[2 * C : 3 * C]

    in_maps = []
    for core in range(N_CORES):
        b = core // 4
        h0 = 4 * (core % 4)
        heads = [h0, h0 + 1, h0 + 2, h0 + 3]
        blocks = []
        for pair in range(2):
            ha, hb = heads[2 * pair], heads[2 * pair + 1]
            blocks.append(
                np.concatenate(
                    [_eo(wq[ha * HD : ha * HD + HD]),
                     _eo(wq[hb * HD : hb * HD + HD])],
                    axis=0,
                )
            )
        for pair in range(2):
            ha, hb = heads[2 * pair], heads[2 * pair + 1]
            blocks.append(
                np.concatenate(
                    [_eo(wk[ha * HD : ha * HD + HD]),
                     _eo(wk[hb * HD : hb * HD + HD])],
                    axis=0,
                )
            )
        wqkT = np.ascontiguousarray(
            np.concatenate(blocks, axis=0).T
        ).astype(f16)  # [C, 512]
        wvT = np.ascontiguousarray(
            wv[h0 * HD : h0 * HD + GC].T
        ).astype(f16)  # [C, 256]
        wpT = np.ascontiguousarray(
            w_proj[:, h0 * HD : h0 * HD + GC].T
        ).astype(f16)  # [256, C]
        in_maps.append(
            {
                "xT": xT[b],
                "wqkT": wqkT,
                "wvT": wvT,
                "wpT": wpT,
                "cc": cc,
                "ss": ss,
                "psw": psw,
                "emat": emat,
                "ident": ident,
                "esel": esel,
            }
        )
    return in_maps


def kernel(x, cos, sin, mask, w_qkv, w_proj, _trace=False, _tmpdir=None):
    x = np.asarray(x, dtype=np.float32)
    cos = np.asarray(cos, dtype=np.float32)
    sin = np.asarray(sin, dtype=np.float32)
    w_qkv = np.asarray(w_qkv, dtype=np.float32)
    w_proj = np.asarray(w_proj, dtype=np.float32)
    # mask is all-ones in this problem spec: no-op in the math.

    nc = _get_program()
    in_maps = _host_prep(x, cos, sin, w_qkv, w_proj)
    res = run_bass_kernel_spmd(
        nc, in_maps, list(range(N_CORES)), trace=_trace, tmpdir=_tmpdir
    )
    out = np.empty((B, T, C), dtype=np.float32)
    for b in range(B):
        acc = res.results[4 * b]["y"].astype(np.float32).copy()
        for g in range(1, 4):
            acc += res.results[4 * b + g]["y"]
        out[b] = acc
    kernel._last_exec_time_ns = res.exec_time_ns
    return out
